# revision 14
# baseline (speedup 1.0000x reference)
"""Cross-attention Trainium2 kernel (8 NeuronCores, SPMD over Q rows).

Math:  out = softmax((m1 Wq^T + bq)(m2 Wk^T + bk)^T / sqrt(H)) (m2 Wv^T + bv)

v2 restructure: both projection matmuls fold into host-precomputed streams so
the device runs ONLY the two N1*N2*dim matmuls plus softmax bookkeeping:
    GT   = scale * Wq^T Wk               (host)
    M2GT = GT @ m2^T          [E, N2]    (host; replaces the C prologue)
    V    = m2 @ Wv^T          [N2, H]    (host; replaces the WvT epilogue)
    d2   = scale * m2 (Wk^T bq) - shift  (host row bias; exact, bk term is
                                          softmax-invariant and dropped)
    per block of kb k-rows:
      ST  = M2GT_blk^T-tiles @ m1T       [kb, QCc]   scores^T
      EST = exp(ST + d2)                              (Act engine)
      s  += ones^T EST                   (PSUM chains held across all blocks)
      OT += V_blk^T-tiles @ EST          [H, QCc]     out^T accumulation
    out^T = OT * (1/s  broadcast via K=1 matmul)  -> DRAM [H, QCc]
    host transposes per-core outputs back to [QCc, H].

Per-core FLOPs = total/8 with zero collectives and no on-device transposes.
ST/OT loops pair the two q chunks per stationary operand so the PE reloads
weights half as often; bulk loads are 2 rearranged DMAs per tensor per block.
"""

import numpy as np

E = 1024
H = 1024
N1 = 8192
N2 = 8192
NCORES = 8
SCALE = 1.0 / np.sqrt(np.float32(H))

_BUILD_CACHE = {}


def _build(mm="f16", biased_q=False, biased_v=False,
           e=E, h=H, n2=N2, qc=N1 // NCORES, kb=1024, stage="full", repeat=1,
           st_bufs=3, ut_bufs=4, s_bufs=1, est_bufs=2):
    """Build (and finalize) the per-core Bass kernel. Returns nc."""
    import concourse.bacc as bacc
    import concourse.tile as tile
    import concourse.mybir as mybir

    f32 = mybir.dt.float32
    rdt = {"f32": f32, "f32r": mybir.dt.float32r, "f16": mybir.dt.float16}[mm]
    shift = 5.0 if mm == "f16" else 0.0   # softmax-invariant; keeps exp in fp16 range

    et = e // 128          # e tiles (contraction tiles for ST)
    ht = h // 128          # h tiles (output rows of OT)
    ktb = kb // 128        # k tiles per block
    nb = n2 // kb          # k blocks
    qw = 512 if qc % 512 == 0 else qc    # q chunk width (matmul N)
    qch = qc // qw
    EXP = mybir.ActivationFunctionType.Exp
    do_sum = stage == "full"
    do_ot = stage == "full"

    nc = bacc.Bacc(None, target_bir_lowering=False)

    m1t_d = nc.dram_tensor("m1t", [e, qc], rdt, kind="ExternalInput")
    m2gt_d = nc.dram_tensor("m2gt", [e, n2], rdt, kind="ExternalInput")
    v_d = nc.dram_tensor("v", [n2, h], rdt, kind="ExternalInput")
    d2_d = nc.dram_tensor("d2", [n2, 1], f32, kind="ExternalInput") if biased_q else None
    bv_d = nc.dram_tensor("bv", [h, 1], f32, kind="ExternalInput") if biased_v else None
    out_d = nc.dram_tensor("out", [h, qc], f32, kind="ExternalOutput")

    import contextlib
    with tile.TileContext(nc) as tc:
        rep_ctx = tc.For_i(0, repeat, 1) if repeat > 1 else contextlib.nullcontext()
        with rep_ctx, tc.tile_pool(name="res", bufs=1) as res, \
                tc.tile_pool(name="m1tp", bufs=2) as m1tp:
            if do_sum:
                ones = res.tile([128, 1], rdt)
                onesrow = res.tile([1, 128], f32)
                nc.vector.memset(onesrow, 1.0)
                if mm == "f32r":
                    ones_f = res.tile([128, 1], f32)
                    nc.vector.memset(ones_f, 1.0)
                    nc.vector.tensor_copy(ones, ones_f)
                else:
                    nc.vector.memset(ones, 1.0)
                recip_row = res.tile([1, qc], f32)
            if shift and not biased_q:
                shift_sb = res.tile([128, 1], f32)
                nc.vector.memset(shift_sb, -shift)
            if do_ot:
                ot_sb = res.tile([128, ht, qc], f32)
            if biased_v:
                bv_sb = res.tile([128, ht], f32)
                nc.sync.dma_start(
                    out=bv_sb,
                    in_=bv_d[:, :].rearrange("(t p) o -> p (t o)", p=128))

            # m1t double-buffers across repeat iterations so the reload
            # prefetches during the previous iteration's main loop.
            m1t_sb = m1tp.tile([128, et, qc], rdt, tag="m1t")
            for hf in range(2):
                nc.sync.dma_start(
                    out=m1t_sb[:, hf * (et // 2):(hf + 1) * (et // 2), :],
                    in_=m1t_d[hf * (et // 2) * 128:(hf + 1) * (et // 2) * 128, :]
                    .rearrange("(t p) q -> p t q", p=128))

            with (
                tc.tile_pool(name="m2gtp", bufs=2) as m2gtp,
                tc.tile_pool(name="vp", bufs=2) as vp,
                tc.tile_pool(name="estp", bufs=est_bufs) as estp,
                tc.tile_pool(name="obp", bufs=3) as obp,
                tc.tile_pool(name="mmps", bufs=st_bufs, space="PSUM") as mmps,
                tc.tile_pool(name="utps", bufs=ut_bufs, space="PSUM") as utps,
                tc.tile_pool(name="sps", bufs=s_bufs, space="PSUM") as sps,
            ):
                # softmax-denominator PSUM accumulators, held across all blocks
                if do_sum:
                    sp_tile = sps.tile([33, qw], f32, tag="s", name="sp_tile")
                    sp_chunks = [sp_tile[32 * q0:32 * q0 + 1, :]
                                 for q0 in range(qch)]
                rb = None
                for b in range(nb):
                    k0 = b * kb
                    last = b == nb - 1
                    m2gt_blk = m2gtp.tile([128, et, kb], rdt, tag="m2gt")
                    for hf in range(2):
                        nc.sync.dma_start(
                            out=m2gt_blk[:, hf * (et // 2):(hf + 1) * (et // 2), :],
                            in_=m2gt_d[hf * (et // 2) * 128:(hf + 1) * (et // 2) * 128,
                                       k0:k0 + kb]
                            .rearrange("(t p) k -> p t k", p=128))
                    if do_ot:
                        v_blk = vp.tile([128, ktb, h], rdt, tag="v")
                        for hf in range(2):
                            nc.sync.dma_start(
                                out=v_blk[:, hf * (ktb // 2):(hf + 1) * (ktb // 2), :],
                                in_=v_d[k0 + hf * (kb // 2):k0 + (hf + 1) * (kb // 2), :]
                                .rearrange("(j p) h -> p j h", p=128))
                    if biased_q:
                        d2_blk = estp.tile([128, ktb], f32, tag="d2")
                        nc.sync.dma_start(
                            out=d2_blk,
                            in_=d2_d[k0:k0 + kb, 0:1].rearrange("(j p) o -> p (j o)", p=128))
                    est = estp.tile([128, ktb, qc], rdt, tag="est")

                    # scores^T for this block, then exp. q0 innermost:
                    # consecutive matmuls share the stationary m2gt tile,
                    # halving PE weight loads; the qch psum chains stay open
                    # simultaneously on distinct banks.
                    for j in range(ktb):
                        stps = [mmps.tile([128, qw], f32, tag="st", name="stp")
                                for _ in range(qch)]
                        for t in range(et):
                            for q0 in range(qch):
                                nc.tensor.matmul(
                                    stps[q0],
                                    m2gt_blk[:, t, j * 128:(j + 1) * 128],
                                    m1t_sb[:, t, q0 * qw:(q0 + 1) * qw],
                                    start=(t == 0), stop=(t == et - 1),
                                    skip_group_check=True,
                                )
                        for q0 in range(qch):
                            nc.scalar.activation(
                                est[:, j, q0 * qw:(q0 + 1) * qw], stps[q0], EXP,
                                bias=(d2_blk[:, j:j + 1] if biased_q else
                                      (shift_sb if shift else 0.0)), scale=1.0)

                    if not do_ot:
                        continue
                    # OT += V_blk^T-tiles @ EST  (q0 pairs share stationary)
                    for t in range(ht):
                        ups = [utps.tile([128, qw], f32, tag="ut", name="up")
                               for _ in range(qch)]
                        for j in range(ktb):
                            for q0 in range(qch):
                                nc.tensor.matmul(
                                    ups[q0],
                                    v_blk[:, j, t * 128:(t + 1) * 128],
                                    est[:, j, q0 * qw:(q0 + 1) * qw],
                                    start=(j == 0), stop=(j == ktb - 1),
                                    skip_group_check=True,
                                )
                        for q0 in range(qch):
                            dst = ot_sb[:, t, q0 * qw:(q0 + 1) * qw]
                            if b == 0:
                                nc.vector.tensor_copy(dst, ups[q0])
                            else:
                                nc.vector.tensor_add(dst, dst, ups[q0])

                        if t == 0 and do_sum:
                            # softmax denominators: s += ones^T @ EST, PSUM
                            # chains spanning the whole k loop. Emitted after
                            # the first OT tile so the trailing exp of this
                            # block is long done.
                            for j in range(ktb):
                                for q0 in range(qch):
                                    nc.tensor.matmul(
                                        sp_chunks[q0], ones,
                                        est[:, j, q0 * qw:(q0 + 1) * qw],
                                        start=(b == 0 and j == 0),
                                        stop=(last and j == ktb - 1),
                                        skip_group_check=True,
                                    )
                            if last:
                                # 1/s, then broadcast across partitions via a
                                # K=1 fp32 matmul into spare OT-pool banks.
                                for q0 in range(qch):
                                    nc.vector.reciprocal(
                                        recip_row[0:1, q0 * qw:(q0 + 1) * qw],
                                        sp_chunks[q0])
                                rb = [utps.tile([128, qw], f32, tag="ut",
                                                name="rb")
                                      for _ in range(qch)]
                                for q0 in range(qch):
                                    nc.tensor.matmul(
                                        rb[q0], onesrow,
                                        recip_row[0:1, q0 * qw:(q0 + 1) * qw],
                                        start=True, stop=True,
                                        skip_group_check=True,
                                    )

                # normalize + store out^T
                if do_ot:
                    for t in range(ht):
                        for q0 in range(qch):
                            ob = obp.tile([128, qw], f32, tag="ob")
                            nc.vector.tensor_mul(
                                ob, ot_sb[:, t, q0 * qw:(q0 + 1) * qw], rb[q0])
                            if biased_v:
                                nc.vector.tensor_scalar_add(
                                    ob, ob, bv_sb[:, t:t + 1])
                            nc.sync.dma_start(
                                out=out_d[t * 128:(t + 1) * 128,
                                          q0 * qw:(q0 + 1) * qw],
                                in_=ob)
                elif stage == "st":
                    # keep the NEFF valid: store the last est tile
                    nc.sync.dma_start(out=out_d[0:128, 0:qc],
                                      in_=est[:, 0, :].bitcast(f32)[:, 0:qc])

    nc.finalize()
    return nc


def _get_nc(key):
    if key not in _BUILD_CACHE:
        _BUILD_CACHE[key] = _build(*key[0], **dict(key[1]))
    return _BUILD_CACHE[key]


def _prep_inputs(molecule1, molecule2, Wq, bq, Wk, bk, Wv, bv, mm="f16"):
    """Host-side prep. Returns (in_maps, biased_q, biased_v)."""
    m1 = np.asarray(molecule1, np.float32)
    m2 = np.ascontiguousarray(np.asarray(molecule2, np.float32))
    wq = np.asarray(Wq, np.float64)
    wk = np.asarray(Wk, np.float64)
    wv = np.asarray(Wv, np.float32)
    bq64 = np.asarray(bq, np.float64)
    bv32 = np.asarray(bv, np.float32)

    scale = 1.0 / np.sqrt(np.float64(wq.shape[0]))
    gt = (scale * (wq.T @ wk)).astype(np.float32)
    m2gt = np.ascontiguousarray(gt @ m2.T)              # [E, N2]
    v = np.ascontiguousarray(m2 @ wv.T)                 # [N2, H]
    m1t = np.ascontiguousarray(m1.T)                    # [E, N1]

    shift = 5.0 if mm == "f16" else 0.0
    v2 = (scale * (wk.T @ bq64)).astype(np.float32)
    biased_q = bool(np.any(v2))
    biased_v = bool(np.any(bv32))
    d2 = (m2 @ v2 - shift).astype(np.float32)           # [N2]

    if mm == "f16":
        cast = lambda a: a.astype(np.float16)
    else:
        cast = lambda a: a
    m2gtc = cast(m2gt)
    vc = cast(v)

    qc = m1.shape[0] // NCORES
    in_maps = []
    for c in range(NCORES):
        m = {
            "m1t": cast(np.ascontiguousarray(m1t[:, c * qc:(c + 1) * qc])),
            "m2gt": m2gtc,
            "v": vc,
        }
        if biased_q:
            m["d2"] = d2.reshape(-1, 1)
        if biased_v:
            m["bv"] = bv32.reshape(-1, 1)
        in_maps.append(m)
    return in_maps, biased_q, biased_v


def kernel(molecule1, molecule2, Wq, bq, Wk, bk, Wv, bv):
    from concourse.bass_utils import run_bass_kernel_spmd

    import os
    mm = os.environ.get("BASS_MM", "f16")
    in_maps, biased_q, biased_v = _prep_inputs(
        molecule1, molecule2, Wq, bq, Wk, bk, Wv, bv, mm=mm)
    kb = 1024 if mm == "f16" else 512
    key = ((mm, biased_q, biased_v), (("kb", kb),))
    nc = _get_nc(key)
    res = run_bass_kernel_spmd(nc, in_maps, core_ids=list(range(NCORES)))
    out = np.concatenate(
        [np.ascontiguousarray(res.results[c]["out"].T) for c in range(NCORES)],
        axis=0)
    return out.astype(np.asarray(molecule1).dtype, copy=False)


# revision 15
# speedup vs baseline: 1.0082x; 1.0082x over previous
"""Cross-attention Trainium2 kernel (8 NeuronCores, SPMD over Q rows).

Math:  out = softmax((m1 Wq^T + bq)(m2 Wk^T + bk)^T / sqrt(H)) (m2 Wv^T + bv)

v2 restructure: both projection matmuls fold into host-precomputed streams so
the device runs ONLY the two N1*N2*dim matmuls plus softmax bookkeeping:
    GT   = scale * Wq^T Wk               (host)
    M2GT = GT @ m2^T          [E, N2]    (host; replaces the C prologue)
    V    = m2 @ Wv^T          [N2, H]    (host; replaces the WvT epilogue)
    d2   = scale * m2 (Wk^T bq) - shift  (host row bias; exact, bk term is
                                          softmax-invariant and dropped)
    per block of kb k-rows:
      ST  = M2GT_blk^T-tiles @ m1T       [kb, QCc]   scores^T
      EST = exp(ST + d2)                              (Act engine)
      s  += ones^T EST                   (PSUM chains held across all blocks)
      OT += V_blk^T-tiles @ EST          [H, QCc]     out^T accumulation
    out^T = OT * (1/s  broadcast via K=1 matmul)  -> DRAM [H, QCc]
    host transposes per-core outputs back to [QCc, H].

Per-core FLOPs = total/8 with zero collectives and no on-device transposes.
ST/OT loops pair the two q chunks per stationary operand so the PE reloads
weights half as often; bulk loads are 2 rearranged DMAs per tensor per block.
"""

import numpy as np

E = 1024
H = 1024
N1 = 8192
N2 = 8192
NCORES = 8
SCALE = 1.0 / np.sqrt(np.float32(H))

_BUILD_CACHE = {}


def _build(mm="f16", biased_q=False, biased_v=False,
           e=E, h=H, n2=N2, qc=N1 // NCORES, kb=1024, stage="full", repeat=1,
           st_bufs=3, ut_bufs=4, s_bufs=1, est_bufs=2):
    """Build (and finalize) the per-core Bass kernel. Returns nc."""
    import concourse.bacc as bacc
    import concourse.tile as tile
    import concourse.mybir as mybir

    f32 = mybir.dt.float32
    rdt = {"f32": f32, "f32r": mybir.dt.float32r, "f16": mybir.dt.float16}[mm]
    shift = 5.0 if mm == "f16" else 0.0   # softmax-invariant; keeps exp in fp16 range

    et = e // 128          # e tiles (contraction tiles for ST)
    ht = h // 128          # h tiles (output rows of OT)
    ktb = kb // 128        # k tiles per block
    nb = n2 // kb          # k blocks
    qw = 512 if qc % 512 == 0 else qc    # q chunk width (matmul N)
    qch = qc // qw
    EXP = mybir.ActivationFunctionType.Exp
    rdt32r = mybir.dt.float32r
    do_sum = stage == "full"
    do_ot = stage == "full"

    nc = bacc.Bacc(None, target_bir_lowering=False)

    m1t_d = nc.dram_tensor("m1t", [e, qc], rdt, kind="ExternalInput")
    m2gt_d = nc.dram_tensor("m2gt", [e, n2], rdt, kind="ExternalInput")
    v_d = nc.dram_tensor("v", [n2, h], rdt, kind="ExternalInput")
    d2_d = nc.dram_tensor("d2", [n2, 1], f32, kind="ExternalInput") if biased_q else None
    bv_d = nc.dram_tensor("bv", [h, 1], f32, kind="ExternalInput") if biased_v else None
    out_d = nc.dram_tensor("out", [h, qc], f32, kind="ExternalOutput")

    import contextlib
    with tile.TileContext(nc) as tc:
        rep_ctx = tc.For_i(0, repeat, 1) if repeat > 1 else contextlib.nullcontext()
        with rep_ctx, tc.tile_pool(name="res", bufs=1) as res, \
                tc.tile_pool(name="m1tp", bufs=2) as m1tp:
            if do_sum:
                ones16 = res.tile([128, 1], rdt)
                nc.vector.memset(ones16, 1.0)
                onesrow = res.tile([1, 128], f32)
                nc.vector.memset(onesrow, 1.0)
                recip_row = res.tile([1, qc], f32)
                sacc = res.tile([128, qc], f32)
            if shift and not biased_q:
                shift_sb = res.tile([128, 1], f32)
                nc.vector.memset(shift_sb, -shift)
            if do_ot:
                ot_sb = res.tile([128, ht, qc], f32)
            if biased_v:
                bv_sb = res.tile([128, ht], f32)
                nc.sync.dma_start(
                    out=bv_sb,
                    in_=bv_d[:, :].rearrange("(t p) o -> p (t o)", p=128))

            # m1t double-buffers across repeat iterations so the reload
            # prefetches during the previous iteration's main loop.
            m1t_sb = m1tp.tile([128, et, qc], rdt, tag="m1t")
            for hf in range(2):
                nc.sync.dma_start(
                    out=m1t_sb[:, hf * (et // 2):(hf + 1) * (et // 2), :],
                    in_=m1t_d[hf * (et // 2) * 128:(hf + 1) * (et // 2) * 128, :]
                    .rearrange("(t p) q -> p t q", p=128))

            with (
                tc.tile_pool(name="m2gtp", bufs=2) as m2gtp,
                tc.tile_pool(name="vp", bufs=2) as vp,
                tc.tile_pool(name="estp", bufs=est_bufs) as estp,
                tc.tile_pool(name="obp", bufs=3) as obp,
                tc.tile_pool(name="mmps", bufs=st_bufs, space="PSUM") as mmps,
                tc.tile_pool(name="utps", bufs=ut_bufs, space="PSUM") as utps,
                tc.tile_pool(name="sps", bufs=s_bufs, space="PSUM") as sps,
            ):
                # softmax-denominator PSUM accumulators, held across all blocks
                if do_sum:
                    sp_tile = sps.tile([33, qw], f32, tag="s", name="sp_tile")
                    sp_chunks = [sp_tile[32 * q0:32 * q0 + 1, :]
                                 for q0 in range(qch)]
                rb = None
                for b in range(nb):
                    k0 = b * kb
                    last = b == nb - 1
                    m2gt_blk = m2gtp.tile([128, et, kb], rdt, tag="m2gt")
                    for hf in range(2):
                        nc.sync.dma_start(
                            out=m2gt_blk[:, hf * (et // 2):(hf + 1) * (et // 2), :],
                            in_=m2gt_d[hf * (et // 2) * 128:(hf + 1) * (et // 2) * 128,
                                       k0:k0 + kb]
                            .rearrange("(t p) k -> p t k", p=128))
                    if do_ot:
                        v_blk = vp.tile([128, ktb, h], rdt, tag="v")
                        for hf in range(2):
                            nc.sync.dma_start(
                                out=v_blk[:, hf * (ktb // 2):(hf + 1) * (ktb // 2), :],
                                in_=v_d[k0 + hf * (kb // 2):k0 + (hf + 1) * (kb // 2), :]
                                .rearrange("(j p) h -> p j h", p=128))
                    if biased_q:
                        d2_blk = estp.tile([128, ktb], f32, tag="d2")
                        nc.sync.dma_start(
                            out=d2_blk,
                            in_=d2_d[k0:k0 + kb, 0:1].rearrange("(j p) o -> p (j o)", p=128))
                    est = estp.tile([128, ktb, qc], rdt, tag="est")

                    # scores^T for this block, then exp. q0 innermost:
                    # consecutive matmuls share the stationary m2gt tile,
                    # halving PE weight loads; the qch psum chains stay open
                    # simultaneously on distinct banks.
                    for j in range(ktb):
                        stps = [mmps.tile([128, qw], f32, tag="st", name="stp")
                                for _ in range(qch)]
                        for t in range(et):
                            for q0 in range(qch):
                                nc.tensor.matmul(
                                    stps[q0],
                                    m2gt_blk[:, t, j * 128:(j + 1) * 128],
                                    m1t_sb[:, t, q0 * qw:(q0 + 1) * qw],
                                    start=(t == 0), stop=(t == et - 1),
                                    skip_group_check=True,
                                )
                        for q0 in range(qch):
                            nc.scalar.activation(
                                est[:, j, q0 * qw:(q0 + 1) * qw], stps[q0], EXP,
                                bias=(d2_blk[:, j:j + 1] if biased_q else
                                      (shift_sb if shift else 0.0)), scale=1.0)

                    if not do_ot:
                        continue
                    # OT += V_blk^T-tiles @ EST  (q0 pairs share stationary)
                    for t in range(ht):
                        ups = [utps.tile([128, qw], f32, tag="ut", name="up")
                               for _ in range(qch)]
                        for j in range(ktb):
                            for q0 in range(qch):
                                nc.tensor.matmul(
                                    ups[q0],
                                    v_blk[:, j, t * 128:(t + 1) * 128],
                                    est[:, j, q0 * qw:(q0 + 1) * qw],
                                    start=(j == 0), stop=(j == ktb - 1),
                                    skip_group_check=True,
                                )
                        for q0 in range(qch):
                            dst = ot_sb[:, t, q0 * qw:(q0 + 1) * qw]
                            if b == 0:
                                nc.vector.tensor_copy(dst, ups[q0])
                            else:
                                nc.vector.tensor_add(dst, dst, ups[q0])

                    if do_sum:
                        # Softmax denominators without streaming EST through
                        # the PE again: fold est pairwise along j on the DVE
                        # (in place, after OT consumed it) and accumulate the
                        # per-partition partial sums in f32; two matmuls at
                        # the very end collapse the partition dim.
                        stride = ktb // 2
                        while stride >= 1:
                            for i in range(stride):
                                nc.vector.tensor_add(
                                    est[:, i, :], est[:, i, :],
                                    est[:, i + stride, :])
                            stride //= 2
                        if b == 0:
                            nc.vector.tensor_copy(sacc, est[:, 0, :])
                        else:
                            nc.vector.tensor_add(sacc, sacc, est[:, 0, :])
                        if last:
                            sacc16 = estp.tile([128, qc], rdt, tag="sacc16")
                            nc.vector.tensor_copy(sacc16, sacc)
                            for q0 in range(qch):
                                nc.tensor.matmul(
                                    sp_chunks[q0], ones16,
                                    sacc16[:, q0 * qw:(q0 + 1) * qw],
                                    start=True, stop=True,
                                    skip_group_check=True,
                                )
                            # 1/s, then broadcast across partitions via a
                            # K=1 fp32 matmul into spare OT-pool banks.
                            for q0 in range(qch):
                                nc.vector.reciprocal(
                                    recip_row[0:1, q0 * qw:(q0 + 1) * qw],
                                    sp_chunks[q0])
                            rb = [utps.tile([128, qw], f32, tag="ut",
                                            name="rb")
                                  for _ in range(qch)]
                            for q0 in range(qch):
                                nc.tensor.matmul(
                                    rb[q0], onesrow,
                                    recip_row[0:1, q0 * qw:(q0 + 1) * qw],
                                    start=True, stop=True,
                                    skip_group_check=True,
                                )

                # normalize + store out^T
                if do_ot:
                    for t in range(ht):
                        for q0 in range(qch):
                            ob = obp.tile([128, qw], f32, tag="ob")
                            nc.vector.tensor_mul(
                                ob, ot_sb[:, t, q0 * qw:(q0 + 1) * qw], rb[q0])
                            if biased_v:
                                nc.vector.tensor_scalar_add(
                                    ob, ob, bv_sb[:, t:t + 1])
                            nc.sync.dma_start(
                                out=out_d[t * 128:(t + 1) * 128,
                                          q0 * qw:(q0 + 1) * qw],
                                in_=ob)
                elif stage == "st":
                    # keep the NEFF valid: store the last est tile
                    nc.sync.dma_start(out=out_d[0:128, 0:qc],
                                      in_=est[:, 0, :].bitcast(f32)[:, 0:qc])

    nc.finalize()
    return nc


def _get_nc(key):
    if key not in _BUILD_CACHE:
        _BUILD_CACHE[key] = _build(*key[0], **dict(key[1]))
    return _BUILD_CACHE[key]


def _prep_inputs(molecule1, molecule2, Wq, bq, Wk, bk, Wv, bv, mm="f16"):
    """Host-side prep. Returns (in_maps, biased_q, biased_v)."""
    m1 = np.asarray(molecule1, np.float32)
    m2 = np.ascontiguousarray(np.asarray(molecule2, np.float32))
    wq = np.asarray(Wq, np.float64)
    wk = np.asarray(Wk, np.float64)
    wv = np.asarray(Wv, np.float32)
    bq64 = np.asarray(bq, np.float64)
    bv32 = np.asarray(bv, np.float32)

    scale = 1.0 / np.sqrt(np.float64(wq.shape[0]))
    gt = (scale * (wq.T @ wk)).astype(np.float32)
    m2gt = np.ascontiguousarray(gt @ m2.T)              # [E, N2]
    v = np.ascontiguousarray(m2 @ wv.T)                 # [N2, H]
    m1t = np.ascontiguousarray(m1.T)                    # [E, N1]

    shift = 5.0 if mm == "f16" else 0.0
    v2 = (scale * (wk.T @ bq64)).astype(np.float32)
    biased_q = bool(np.any(v2))
    biased_v = bool(np.any(bv32))
    d2 = (m2 @ v2 - shift).astype(np.float32)           # [N2]

    if mm == "f16":
        cast = lambda a: a.astype(np.float16)
    else:
        cast = lambda a: a
    m2gtc = cast(m2gt)
    vc = cast(v)

    qc = m1.shape[0] // NCORES
    in_maps = []
    for c in range(NCORES):
        m = {
            "m1t": cast(np.ascontiguousarray(m1t[:, c * qc:(c + 1) * qc])),
            "m2gt": m2gtc,
            "v": vc,
        }
        if biased_q:
            m["d2"] = d2.reshape(-1, 1)
        if biased_v:
            m["bv"] = bv32.reshape(-1, 1)
        in_maps.append(m)
    return in_maps, biased_q, biased_v


def kernel(molecule1, molecule2, Wq, bq, Wk, bk, Wv, bv):
    from concourse.bass_utils import run_bass_kernel_spmd

    import os
    mm = os.environ.get("BASS_MM", "f16")
    in_maps, biased_q, biased_v = _prep_inputs(
        molecule1, molecule2, Wq, bq, Wk, bk, Wv, bv, mm=mm)
    kb = 1024 if mm == "f16" else 512
    key = ((mm, biased_q, biased_v), (("kb", kb),))
    nc = _get_nc(key)
    res = run_bass_kernel_spmd(nc, in_maps, core_ids=list(range(NCORES)))
    out = np.concatenate(
        [np.ascontiguousarray(res.results[c]["out"].T) for c in range(NCORES)],
        axis=0)
    return out.astype(np.asarray(molecule1).dtype, copy=False)
